# revision 17
# baseline (speedup 1.0000x reference)
"""Bahdanau attention on 8 Trainium2 NeuronCores (Bass/Tile).

Problem:  S=2048, B=32, D=1024, E2=1024
  ws  = dec @ Wb.T                       [B, D]
  WH  = enc @ Wc.T                       [S, B, D]
  sc  = tanh(WH + ws) . Wa               [S, B]
  at  = softmax(sc, axis=0)              [S, B]
  out = einsum('sb,sbe->be', at, enc)[None]   [1, B, 2E]

Sharding: data-parallel over batch B across 8 cores (4 batches/core);
Wb/Wc/Wa replicated. Softmax axis (S) stays core-local.

Per-core dataflow (all matmuls in fp32r = TF32-precision, 1 col/cycle):
  - enc tiles loaded natural [s'=128, e] with an fp32->fp32r cast in-DMA
  - PE-transpose -> encT [e, s'] chunks (fp32r, 1.5 cyc/col)
  - WH^T [d-chunk=128p, s'] = (WcT chunk).T @ encT, accumulated over e
  - ACT: tanh(WH + wsT[d,b]) fused via per-partition bias
  - score row [1, s'] = sum_d WaChunk.T @ tanh_chunk  (PE)
  - exp on ACT (no max subtraction: |score| <= sum|Wa| ~ 26, exp fits fp32
    comfortably and softmax is shift-invariant), Z via activation accum_out
  - context [1, e] += expT.T @ enc_nat on PE across all s-tiles (unnormalized),
    scaled by 1/Z once per batch at the end.
"""

import numpy as np

S, B, D, E2 = 2048, 32, 1024, 1024
NCORES = 8
BL = B // NCORES          # batches per core
ST = 512                  # s-tile size
NST = S // ST             # s-tiles per batch
NSUB = ST // 128          # 128-row subtiles per s-tile
EC = E2 // 128            # e chunks
DC = D // 128             # d chunks

_CACHE = {}


def _build_nc():
    import concourse.bacc as bacc
    import concourse.tile as tile
    from concourse import mybir
    from concourse.masks import make_identity

    f32 = mybir.dt.float32
    f32r = mybir.dt.float32r
    TANH = mybir.ActivationFunctionType.Tanh
    EXP = mybir.ActivationFunctionType.Exp
    X = mybir.AxisListType.X

    nc = bacc.Bacc()
    enc = nc.declare_dram_parameter("enc", [S, BL, E2], f32, isOutput=False)
    dect = nc.declare_dram_parameter("dect", [D, BL], f32, isOutput=False)
    wbt = nc.declare_dram_parameter("wbt", [D, D], f32, isOutput=False)    # Wb.T [d, d2]
    wct = nc.declare_dram_parameter("wct", [E2, D], f32, isOutput=False)   # Wc.T [e, d]
    wa2 = nc.declare_dram_parameter("wa2", [128, DC], f32, isOutput=False) # Wa chunks as cols
    outp = nc.declare_dram_parameter("out", [BL, E2], f32, isOutput=True)

    with tile.TileContext(nc) as tc:
        with (
            tc.tile_pool(name="const", bufs=1) as const_pool,
            tc.tile_pool(name="wbtp", bufs=3) as wbt_pool,
            tc.tile_pool(name="encn", bufs=8) as encn_pool,
            tc.tile_pool(name="enct", bufs=16) as enct_pool,
            tc.tile_pool(name="tanhp", bufs=4) as tanh_pool,
            tc.tile_pool(name="rows", bufs=2) as row_pool,
            tc.tile_pool(name="tp_ps", bufs=3, space="PSUM") as tp_ps,
            tc.tile_pool(name="wh_ps", bufs=2, space="PSUM") as wh_ps,
            tc.tile_pool(name="sc_ps", bufs=1, space="PSUM") as sc_ps,
            tc.tile_pool(name="ctx_ps", bufs=2, space="PSUM") as ctx_ps,
        ):
            ex_ps = tp_ps  # columnize psum shares the transpose-scratch banks

            # ---- constants ----
            id32 = const_pool.tile([128, 128], f32)
            make_identity(nc, id32)
            ident = const_pool.tile([128, 128], f32r)
            nc.vector.tensor_copy(out=ident, in_=id32)

            # First compute tile's enc loads go in front of the (8 MB of)
            # weight DMAs so the PE pipeline starts ~20us earlier.
            subs_cache = {}
            first_subs = []
            for j in range(NSUB):
                sub_t = encn_pool.tile([128, E2], f32r, tag="sub", name=f"sub0_{j}")
                nc.gpsimd.dma_start(out=sub_t, in_=enc[j * 128:(j + 1) * 128, 0, :])
                first_subs.append(sub_t)
            subs_cache[(0, 0)] = first_subs

            wct_sb = []
            for ecn in range(EC):
                t = const_pool.tile([128, D], f32r, tag="wct_sb", bufs=EC)
                nc.gpsimd.dma_start(out=t, in_=wct[ecn * 128:(ecn + 1) * 128, :])
                wct_sb.append(t)

            wa_sb = const_pool.tile([128, DC], f32r)
            nc.gpsimd.dma_start(out=wa_sb, in_=wa2[:, :])

            dect_sb = []
            for dk in range(DC):
                t = const_pool.tile([128, BL], f32r, tag="dect_sb", bufs=DC)
                nc.gpsimd.dma_start(out=t, in_=dect[dk * 128:(dk + 1) * 128, :])
                dect_sb.append(t)

            # ---- ws = dec @ Wb.T  ->  wsT [d2-chunk, b] for tanh bias ----
            # ws natural [BL, d2]: lhsT = dectChunk [dk, BL], rhs = wbtChunk [dk, d2]
            ws_sb = const_pool.tile([BL, D], f32r)
            wbt_sb = []
            for dk in range(DC):
                t = wbt_pool.tile([128, D], f32r, tag="wbt_sb", bufs=DC, name=f"wbt{dk}")
                nc.gpsimd.dma_start(out=t, in_=wbt[dk * 128:(dk + 1) * 128, :])
                wbt_sb.append(t)
            for eh in range(2):
                ws_psum = wh_ps.tile([BL, 512], f32, tag="wh")
                for dk in range(DC):
                    nc.tensor.matmul(
                        ws_psum, dect_sb[dk], wbt_sb[dk][:, eh * 512:(eh + 1) * 512],
                        start=(dk == 0), stop=(dk == DC - 1),
                    )
                nc.scalar.copy(out=ws_sb[:, eh * 512:(eh + 1) * 512], in_=ws_psum)
            # transpose ws -> wst chunks [128, BL]
            wst_sb = []
            for dcn in range(DC):
                tp = tp_ps.tile([128, ST], f32r, tag="tp")
                nc.tensor.transpose(
                    tp[:, 0:BL], ws_sb[0:BL, dcn * 128:(dcn + 1) * 128], ident[0:BL, 0:BL]
                )
                w = const_pool.tile([128, BL], f32, tag="wst_sb", bufs=DC)
                nc.vector.tensor_copy(out=w, in_=tp[:, 0:BL])
                wst_sb.append(w)

            # ---- main loop over (batch, s-tile) ----
            for bj in range(BL):
                # [32, S] so the row->column move can be a K=32 matmul against
                # e0 (rows 1-31 are zero); only row 0 holds exp scores.
                exp_all = row_pool.tile([32, S], f32, tag="exp_all")
                nc.vector.memset(exp_all, 0.0)
                zparts = row_pool.tile([1, NST], f32, tag="zparts")
                ctx = [ctx_ps.tile([1, 512], f32, tag="ctx", name=f"ctx{eh}")
                       for eh in range(2)]

                for st in range(NST):
                    s0 = st * ST
                    # 1) natural enc subtiles [128, E2] (fp32r cast in DMA)
                    subs = subs_cache.pop((bj, st), None)
                    if subs is None:
                        subs = []
                        for j in range(NSUB):
                            sub_t = encn_pool.tile([128, E2], f32r, tag="sub")
                            nc.gpsimd.dma_start(
                                out=sub_t,
                                in_=enc[s0 + j * 128:s0 + (j + 1) * 128, bj, :],
                            )
                            subs.append(sub_t)

                    # 2) transpose -> encT [e-chunk=128, ST]
                    enct = []
                    for ecn in range(EC):
                        tp = tp_ps.tile([128, ST], f32r, tag="tp")
                        for j in range(NSUB):
                            nc.tensor.transpose(
                                tp[:, j * 128:(j + 1) * 128],
                                subs[j][:, ecn * 128:(ecn + 1) * 128],
                                ident,
                            )
                        et = enct_pool.tile([128, ST], f32r, tag="et")
                        if ecn % 4 == 0:  # ACT is tanh-heavy; DVE takes most copies
                            nc.scalar.copy(out=et, in_=tp)
                        else:
                            nc.vector.tensor_copy(out=et, in_=tp)
                        enct.append(et)

                    # 3) WH^T + tanh + score, d-chunks in pairs (2 psum banks)
                    sc = sc_ps.tile([1, ST], f32, tag="sc")
                    for dp in range(DC // 2):
                        whs = [wh_ps.tile([128, ST], f32, tag="wh", name=f"wh{dd}")
                               for dd in range(2)]
                        for ecn in range(EC):
                            for dd in range(2):
                                dcn = dp * 2 + dd
                                nc.tensor.matmul(
                                    whs[dd],
                                    wct_sb[ecn][:, dcn * 128:(dcn + 1) * 128],
                                    enct[ecn],
                                    start=(ecn == 0), stop=(ecn == EC - 1),
                                )
                        for dd in range(2):
                            dcn = dp * 2 + dd
                            th = tanh_pool.tile([128, ST], f32r, tag="th")
                            nc.scalar.activation(
                                out=th, in_=whs[dd], func=TANH,
                                bias=wst_sb[dcn][:, bj:bj + 1], scale=1.0,
                            )
                            nc.tensor.matmul(
                                sc, wa_sb[:, dcn:dcn + 1], th,
                                start=(dcn == 0), stop=(dcn == DC - 1),
                                skip_group_check=True,
                            )

                    # 4) exp (+ per-tile partial of Z via accum_out)
                    nc.scalar.activation(
                        out=exp_all[0:1, s0:s0 + ST], in_=sc, func=EXP,
                        accum_out=zparts[0:1, st:st + 1],
                    )

                    # 5) exp row -> columns [128, 1]: K=32 fp32 matmul with e0
                    #    (rows 1-31 of exp_all are zero, so out = row 0)
                    ex = ex_ps.tile([128, NSUB], f32, tag="tp")
                    for j in range(NSUB):
                        nc.tensor.matmul(
                            ex[:, j:j + 1],
                            exp_all[0:32, s0 + j * 128:s0 + (j + 1) * 128],
                            id32[0:32, 0:1],
                            start=True, stop=True,
                        )
                    ext = row_pool.tile([128, NSUB], f32r, tag="ext", bufs=3)
                    nc.vector.tensor_copy(out=ext, in_=ex)

                    # 6) unnormalized context accumulation over all s of this b
                    for j in range(NSUB):
                        for eh in range(2):
                            nc.tensor.matmul(
                                ctx[eh],
                                ext[:, j:j + 1],
                                subs[j][:, eh * 512:(eh + 1) * 512],
                                start=(st == 0 and j == 0),
                                stop=(st == NST - 1 and j == NSUB - 1),
                                skip_group_check=True,
                            )

                # ---- finish batch: scale by 1/Z, write out ----
                z = row_pool.tile([1, 1], f32, tag="z")
                nc.vector.reduce_sum(out=z, in_=zparts, axis=X)
                rz = row_pool.tile([1, 1], f32, tag="rz")
                nc.vector.reciprocal(out=rz, in_=z)
                ctx_sb = row_pool.tile([1, E2], f32, tag="ctx_sb")
                for eh in range(2):
                    nc.vector.tensor_scalar_mul(
                        out=ctx_sb[0:1, eh * 512:(eh + 1) * 512],
                        in0=ctx[eh], scalar1=rz,
                    )
                nc.sync.dma_start(out=outp[bj:bj + 1, :], in_=ctx_sb)

    nc.finalize()
    return nc


def _prep_inputs(dec_prev_hidden, enc_outputs, Wb, Wc, Wa):
    dec_prev_hidden = np.ascontiguousarray(np.asarray(dec_prev_hidden, dtype=np.float32))
    enc_outputs = np.ascontiguousarray(np.asarray(enc_outputs, dtype=np.float32))
    Wb = np.asarray(Wb, dtype=np.float32)
    Wc = np.asarray(Wc, dtype=np.float32)
    Wa = np.asarray(Wa, dtype=np.float32)

    wbt = np.ascontiguousarray(Wb.T)                     # [d, d2]
    wct = np.ascontiguousarray(Wc.T)                     # [e, d]
    dect = np.ascontiguousarray(dec_prev_hidden.T)       # [D, B]
    wa2 = np.ascontiguousarray(Wa.reshape(DC, 128).T)    # [128, DC]

    in_maps = []
    for i in range(NCORES):
        bsl = slice(i * BL, (i + 1) * BL)
        in_maps.append({
            "enc": np.ascontiguousarray(enc_outputs[:, bsl, :]),
            "dect": np.ascontiguousarray(dect[:, bsl]),
            "wbt": wbt,
            "wct": wct,
            "wa2": wa2,
        })
    return in_maps


def _run(inputs, trace=False):
    from concourse.bass_utils import run_bass_kernel_spmd

    if "nc" not in _CACHE:
        _CACHE["nc"] = _build_nc()
    nc = _CACHE["nc"]
    in_maps = _prep_inputs(**inputs)
    res = run_bass_kernel_spmd(nc, in_maps, list(range(NCORES)), trace=trace)
    out = np.concatenate([res.results[i]["out"] for i in range(NCORES)], axis=0)
    return out[None, :, :].astype(np.float32), res


def kernel(dec_prev_hidden, enc_outputs, Wb, Wc, Wa):
    out, _ = _run(dict(
        dec_prev_hidden=dec_prev_hidden, enc_outputs=enc_outputs,
        Wb=Wb, Wc=Wc, Wa=Wa,
    ))
    return out


# revision 18
# speedup vs baseline: 1.3357x; 1.3357x over previous
"""Bahdanau attention on 8 Trainium2 NeuronCores (Bass/Tile).

Problem:  S=2048, B=32, D=1024, E2=1024
  ws  = dec @ Wb.T                       [B, D]
  WH  = enc @ Wc.T                       [S, B, D]
  sc  = tanh(WH + ws) . Wa               [S, B]
  at  = softmax(sc, axis=0)              [S, B]
  out = einsum('sb,sbe->be', at, enc)[None]   [1, B, 2E]

Sharding: data-parallel over batch B across 8 cores (4 batches/core);
Wb/Wc/Wa replicated. Softmax axis (S) stays core-local.

Per-core dataflow (all matmuls in fp32r = TF32-precision, 1 col/cycle):
  - enc tiles loaded natural [s'=128, e] with an fp32->fp32r cast in-DMA
  - PE-transpose -> encT [e, s'] chunks (fp32r, 1.5 cyc/col)
  - WH^T [d-chunk=128p, s'] = (WcT chunk).T @ encT, accumulated over e
  - ACT: tanh(WH + wsT[d,b]) fused via per-partition bias
  - score row [1, s'] = sum_d WaChunk.T @ tanh_chunk  (PE)
  - exp on ACT (no max subtraction: |score| <= sum|Wa| ~ 26, exp fits fp32
    comfortably and softmax is shift-invariant), Z via activation accum_out
  - context [1, e] += expT.T @ enc_nat on PE across all s-tiles (unnormalized),
    scaled by 1/Z once per batch at the end.

Engines run their instruction streams in order, so emission order doubles as
a schedule: tile(0,0)'s enc DMAs + transposes are emitted before the ws
(bias) computation to cover the weight-DMA latency at kernel start.
"""

import numpy as np

S, B, D, E2 = 2048, 32, 1024, 1024
NCORES = 8
BL = B // NCORES          # batches per core
ST = 512                  # s-tile size
NST = S // ST             # s-tiles per batch
NSUB = ST // 128          # 128-row subtiles per s-tile
EC = E2 // 128            # e chunks
DC = D // 128             # d chunks

_CACHE = {}


def _build_nc():
    import concourse.bacc as bacc
    import concourse.tile as tile
    from concourse import mybir
    from concourse.masks import make_identity

    f32 = mybir.dt.float32
    f32r = mybir.dt.float32r
    TANH = mybir.ActivationFunctionType.Tanh
    EXP = mybir.ActivationFunctionType.Exp
    X = mybir.AxisListType.X

    nc = bacc.Bacc()
    enc = nc.declare_dram_parameter("enc", [S, BL, E2], f32, isOutput=False)
    dect = nc.declare_dram_parameter("dect", [D, BL], f32, isOutput=False)
    wbt = nc.declare_dram_parameter("wbt", [D, D], f32, isOutput=False)    # Wb.T [d, d2]
    wct = nc.declare_dram_parameter("wct", [E2, D], f32, isOutput=False)   # Wc.T [e, d]
    wa2 = nc.declare_dram_parameter("wa2", [128, DC], f32, isOutput=False) # Wa chunks as cols
    outp = nc.declare_dram_parameter("out", [BL, E2], f32, isOutput=True)

    with tile.TileContext(nc) as tc:
        with (
            tc.tile_pool(name="const", bufs=1) as const_pool,
            tc.tile_pool(name="wbtp", bufs=1) as wbt_pool,
            tc.tile_pool(name="encn", bufs=8) as encn_pool,
            tc.tile_pool(name="enct", bufs=16) as enct_pool,
            tc.tile_pool(name="tanhp", bufs=4) as tanh_pool,
            tc.tile_pool(name="rows", bufs=2) as row_pool,
            tc.tile_pool(name="tp_ps", bufs=2, space="PSUM") as tp_ps,
            tc.tile_pool(name="wh_ps", bufs=2, space="PSUM") as wh_ps,
            tc.tile_pool(name="sc_ps", bufs=1, space="PSUM") as sc_ps,
            tc.tile_pool(name="ctx_ps", bufs=2, space="PSUM") as ctx_ps,
            tc.tile_pool(name="ex_ps", bufs=1, space="PSUM") as ex_ps,
        ):
            # ---- identity (fp32 via gpsimd, cast copy to fp32r) ----
            id32 = const_pool.tile([128, 128], f32)
            make_identity(nc, id32)
            ident = const_pool.tile([128, 128], f32r)
            nc.vector.tensor_copy(out=ident, in_=id32)

            # ---- emission helpers (order == per-engine schedule) ----
            def load_subs(bj, st, pfx=""):
                s0 = st * ST
                subs = []
                for j in range(NSUB):
                    sub_t = encn_pool.tile(
                        [128, E2], f32r, tag="sub", name=f"sub{pfx}_{j}"
                    )
                    nc.gpsimd.dma_start(
                        out=sub_t, in_=enc[s0 + j * 128:s0 + (j + 1) * 128, bj, :]
                    )
                    subs.append(sub_t)
                return subs

            def transpose_tile(subs):
                enct = []
                for ecn in range(EC):
                    tp = tp_ps.tile([128, ST], f32r, tag="tp", name="tp")
                    for j in range(NSUB):
                        nc.tensor.transpose(
                            tp[:, j * 128:(j + 1) * 128],
                            subs[j][:, ecn * 128:(ecn + 1) * 128],
                            ident,
                        )
                    et = enct_pool.tile([128, ST], f32r, tag="et", name="et")
                    if ecn % 4 == 0:  # ACT is tanh-heavy; DVE takes most copies
                        nc.scalar.copy(out=et, in_=tp)
                    else:
                        nc.vector.tensor_copy(out=et, in_=tp)
                    enct.append(et)
                return enct

            # tile(0,0)'s loads + transposes first: PE has work ~8us in,
            # while the weight DMAs (below) stream.
            subs_cache = {(0, 0): load_subs(0, 0, pfx="00")}

            wbt_sb = []
            for dk in range(DC):
                t = wbt_pool.tile([128, D], f32r, tag="wbt_sb", bufs=DC, name=f"wbt{dk}")
                nc.gpsimd.dma_start(out=t, in_=wbt[dk * 128:(dk + 1) * 128, :])
                wbt_sb.append(t)
            dect_sb = []
            for dk in range(DC):
                t = const_pool.tile([128, BL], f32r, tag="dect_sb", bufs=DC, name=f"dect{dk}")
                nc.gpsimd.dma_start(out=t, in_=dect[dk * 128:(dk + 1) * 128, :])
                dect_sb.append(t)
            wct_sb = []
            for ecn in range(EC):
                t = const_pool.tile([128, D], f32r, tag="wct_sb", bufs=EC, name=f"wct{ecn}")
                nc.gpsimd.dma_start(out=t, in_=wct[ecn * 128:(ecn + 1) * 128, :])
                wct_sb.append(t)
            wa_sb = const_pool.tile([128, DC], f32r)
            nc.gpsimd.dma_start(out=wa_sb, in_=wa2[:, :])

            enct_cache = {(0, 0): transpose_tile(subs_cache[(0, 0)])}

            # ---- ws = dec @ Wb.T -> wsT [d2-chunk, b] for the tanh bias ----
            # ws natural [BL, d2]: lhsT = dectChunk [dk, BL], rhs = wbtChunk
            ws_sb = const_pool.tile([BL, D], f32r)
            for eh in range(2):
                ws_psum = wh_ps.tile([BL, 512], f32, tag="wh", name="ws_psum")
                for dk in range(DC):
                    nc.tensor.matmul(
                        ws_psum, dect_sb[dk], wbt_sb[dk][:, eh * 512:(eh + 1) * 512],
                        start=(dk == 0), stop=(dk == DC - 1),
                    )
                nc.scalar.copy(out=ws_sb[:, eh * 512:(eh + 1) * 512], in_=ws_psum)
            wst_sb = []
            for dcn in range(DC):
                tp = tp_ps.tile([128, ST], f32r, tag="tp", name="tp_ws")
                nc.tensor.transpose(
                    tp[:, 0:BL], ws_sb[0:BL, dcn * 128:(dcn + 1) * 128], ident[0:BL, 0:BL]
                )
                w = const_pool.tile([128, BL], f32, tag="wst_sb", bufs=DC, name=f"wst{dcn}")
                nc.vector.tensor_copy(out=w, in_=tp[:, 0:BL])
                wst_sb.append(w)

            # ---- main loop over (batch, s-tile) ----
            for bj in range(BL):
                # [32, S] so the row->column move can be a K=32 matmul against
                # e0 (rows 1-31 are zero); only row 0 holds exp scores.
                exp_all = row_pool.tile([32, S], f32, tag="exp_all")
                nc.vector.memset(exp_all, 0.0)
                zparts = row_pool.tile([1, NST], f32, tag="zparts")
                ctx = [ctx_ps.tile([1, 512], f32, tag="ctx", name=f"ctx{eh}")
                       for eh in range(2)]

                for st in range(NST):
                    s0 = st * ST
                    subs = subs_cache.pop((bj, st), None) or load_subs(bj, st)
                    enct = enct_cache.pop((bj, st), None) or transpose_tile(subs)

                    # WH^T + tanh + score, d-chunks in pairs (2 psum banks)
                    sc = sc_ps.tile([1, ST], f32, tag="sc")
                    for dp in range(DC // 2):
                        whs = [wh_ps.tile([128, ST], f32, tag="wh", name=f"wh{dd}")
                               for dd in range(2)]
                        for ecn in range(EC):
                            for dd in range(2):
                                dcn = dp * 2 + dd
                                nc.tensor.matmul(
                                    whs[dd],
                                    wct_sb[ecn][:, dcn * 128:(dcn + 1) * 128],
                                    enct[ecn],
                                    start=(ecn == 0), stop=(ecn == EC - 1),
                                )
                        for dd in range(2):
                            dcn = dp * 2 + dd
                            th = tanh_pool.tile([128, ST], f32r, tag="th", name="th")
                            nc.scalar.activation(
                                out=th, in_=whs[dd], func=TANH,
                                bias=wst_sb[dcn][:, bj:bj + 1], scale=1.0,
                            )
                            nc.tensor.matmul(
                                sc, wa_sb[:, dcn:dcn + 1], th,
                                start=(dcn == 0), stop=(dcn == DC - 1),
                                skip_group_check=True,
                            )

                    # exp (+ per-tile partial of Z via accum_out)
                    nc.scalar.activation(
                        out=exp_all[0:1, s0:s0 + ST], in_=sc, func=EXP,
                        accum_out=zparts[0:1, st:st + 1],
                    )

                    # exp row -> columns [128, 1]: K=32 fp32 matmul with e0
                    # (rows 1-31 of exp_all are zero, so out = row 0)
                    ex = ex_ps.tile([128, NSUB], f32, tag="ex")
                    for j in range(NSUB):
                        nc.tensor.matmul(
                            ex[:, j:j + 1],
                            exp_all[0:32, s0 + j * 128:s0 + (j + 1) * 128],
                            id32[0:32, 0:1],
                            start=True, stop=True,
                        )
                    ext = row_pool.tile([128, NSUB], f32r, tag="ext", bufs=3)
                    nc.vector.tensor_copy(out=ext, in_=ex)

                    # unnormalized context accumulation over all s of this b
                    for j in range(NSUB):
                        for eh in range(2):
                            nc.tensor.matmul(
                                ctx[eh],
                                ext[:, j:j + 1],
                                subs[j][:, eh * 512:(eh + 1) * 512],
                                start=(st == 0 and j == 0),
                                stop=(st == NST - 1 and j == NSUB - 1),
                                skip_group_check=True,
                            )

                # ---- finish batch: scale by 1/Z, write out ----
                z = row_pool.tile([1, 1], f32, tag="z")
                nc.vector.reduce_sum(out=z, in_=zparts, axis=X)
                rz = row_pool.tile([1, 1], f32, tag="rz")
                nc.vector.reciprocal(out=rz, in_=z)
                ctx_sb = row_pool.tile([1, E2], f32, tag="ctx_sb")
                for eh in range(2):
                    nc.vector.tensor_scalar_mul(
                        out=ctx_sb[0:1, eh * 512:(eh + 1) * 512],
                        in0=ctx[eh], scalar1=rz,
                    )
                nc.sync.dma_start(out=outp[bj:bj + 1, :], in_=ctx_sb)

    nc.finalize()
    return nc


def _prep_inputs(dec_prev_hidden, enc_outputs, Wb, Wc, Wa):
    dec_prev_hidden = np.ascontiguousarray(np.asarray(dec_prev_hidden, dtype=np.float32))
    enc_outputs = np.ascontiguousarray(np.asarray(enc_outputs, dtype=np.float32))
    Wb = np.asarray(Wb, dtype=np.float32)
    Wc = np.asarray(Wc, dtype=np.float32)
    Wa = np.asarray(Wa, dtype=np.float32)

    wbt = np.ascontiguousarray(Wb.T)                     # [d, d2]
    wct = np.ascontiguousarray(Wc.T)                     # [e, d]
    dect = np.ascontiguousarray(dec_prev_hidden.T)       # [D, B]
    wa2 = np.ascontiguousarray(Wa.reshape(DC, 128).T)    # [128, DC]

    in_maps = []
    for i in range(NCORES):
        bsl = slice(i * BL, (i + 1) * BL)
        in_maps.append({
            "enc": np.ascontiguousarray(enc_outputs[:, bsl, :]),
            "dect": np.ascontiguousarray(dect[:, bsl]),
            "wbt": wbt,
            "wct": wct,
            "wa2": wa2,
        })
    return in_maps


def _run(inputs, trace=False):
    from concourse.bass_utils import run_bass_kernel_spmd

    if "nc" not in _CACHE:
        _CACHE["nc"] = _build_nc()
    nc = _CACHE["nc"]
    in_maps = _prep_inputs(**inputs)
    res = run_bass_kernel_spmd(nc, in_maps, list(range(NCORES)), trace=trace)
    out = np.concatenate([res.results[i]["out"] for i in range(NCORES)], axis=0)
    return out[None, :, :].astype(np.float32), res


def kernel(dec_prev_hidden, enc_outputs, Wb, Wc, Wa):
    out, _ = _run(dict(
        dec_prev_hidden=dec_prev_hidden, enc_outputs=enc_outputs,
        Wb=Wb, Wc=Wc, Wa=Wa,
    ))
    return out


# revision 23
# speedup vs baseline: 1.4294x; 1.0701x over previous
"""Bahdanau attention on 8 Trainium2 NeuronCores (Bass/Tile).

Problem:  S=2048, B=32, D=1024, E2=1024
  ws  = dec @ Wb.T                       [B, D]
  WH  = enc @ Wc.T                       [S, B, D]
  sc  = tanh(WH + ws) . Wa               [S, B]
  at  = softmax(sc, axis=0)              [S, B]
  out = einsum('sb,sbe->be', at, enc)[None]   [1, B, 2E]

Sharding: data-parallel over batch B across 8 cores (4 batches/core);
Wb/Wc/Wa replicated. Softmax axis (S) stays core-local.

Per-core dataflow (all matmuls in fp32r = TF32-precision, 1 col/cycle):
  - enc tiles loaded natural [s'=128, e] with an fp32->fp32r cast in-DMA
  - PE-transpose -> encT [e, s'] chunks (fp32r, 1.5 cyc/col)
  - WH^T [d-chunk=128p, s'] = (WcT chunk).T @ encT, accumulated over e
  - ACT: tanh(WH + wsT[d,b]) fused via per-partition bias
  - score row [1, s'] = sum_d WaChunk.T @ tanh_chunk  (PE)
  - exp on ACT (no max subtraction: |score| <= sum|Wa| ~ 26, exp fits fp32
    comfortably and softmax is shift-invariant), Z via activation accum_out
  - context [1, e] += expT.T @ enc_nat on PE across all s-tiles (unnormalized),
    scaled by 1/Z once per batch at the end.

Engines run their instruction streams in order, so emission order doubles as
a schedule: tile(0,0)'s enc DMAs + transposes are emitted before the ws
(bias) computation to cover the weight-DMA latency at kernel start.
"""

import numpy as np

S, B, D, E2 = 2048, 32, 1024, 1024
NCORES = 8
BL = B // NCORES          # batches per core
ST = 512                  # s-tile size
NST = S // ST             # s-tiles per batch
NSUB = ST // 128          # 128-row subtiles per s-tile
EC = E2 // 128            # e chunks
DC = D // 128             # d chunks

_CACHE = {}


def _build_nc():
    import concourse.bacc as bacc
    import concourse.tile as tile
    from concourse import mybir
    from concourse.masks import make_identity

    f32 = mybir.dt.float32
    f32r = mybir.dt.float32r
    TANH = mybir.ActivationFunctionType.Tanh
    EXP = mybir.ActivationFunctionType.Exp
    X = mybir.AxisListType.X

    nc = bacc.Bacc()
    enc = nc.declare_dram_parameter("enc", [S, BL, E2], f32, isOutput=False)
    dect = nc.declare_dram_parameter("dect", [D, BL], f32, isOutput=False)
    wbt = nc.declare_dram_parameter("wbt", [D, D], f32, isOutput=False)    # Wb.T [d, d2]
    wct = nc.declare_dram_parameter("wct", [E2, D], f32, isOutput=False)   # Wc.T [e, d]
    wa2 = nc.declare_dram_parameter("wa2", [128, DC], f32, isOutput=False) # Wa chunks as cols
    outp = nc.declare_dram_parameter("out", [BL, E2], f32, isOutput=True)

    with tile.TileContext(nc) as tc:
        with (
            tc.tile_pool(name="const", bufs=1) as const_pool,
            tc.tile_pool(name="wbtp", bufs=1) as wbt_pool,
            tc.tile_pool(name="encn", bufs=12) as encn_pool,
            tc.tile_pool(name="enct", bufs=16) as enct_pool,
            tc.tile_pool(name="tanhp", bufs=4) as tanh_pool,
            tc.tile_pool(name="rows", bufs=2) as row_pool,
            tc.tile_pool(name="tp_ps", bufs=2, space="PSUM") as tp_ps,
            tc.tile_pool(name="wh_ps", bufs=2, space="PSUM") as wh_ps,
            tc.tile_pool(name="sc_ps", bufs=1, space="PSUM") as sc_ps,
            tc.tile_pool(name="ctx_ps", bufs=2, space="PSUM") as ctx_ps,
            tc.tile_pool(name="ex_ps", bufs=1, space="PSUM") as ex_ps,
        ):
            # ---- identity (fp32 via gpsimd, cast copy to fp32r) ----
            id32 = const_pool.tile([128, 128], f32)
            make_identity(nc, id32)
            ident = const_pool.tile([128, 128], f32r)
            nc.vector.tensor_copy(out=ident, in_=id32)

            # ---- emission helpers (order == per-engine schedule) ----
            def load_subs(bj, st, pfx=""):
                s0 = st * ST
                subs = []
                for j in range(NSUB):
                    sub_t = encn_pool.tile(
                        [128, E2], f32r, tag="sub", name=f"sub{pfx}_{j}"
                    )
                    nc.gpsimd.dma_start(
                        out=sub_t, in_=enc[s0 + j * 128:s0 + (j + 1) * 128, bj, :]
                    )
                    subs.append(sub_t)
                return subs

            def transpose_tile(subs):
                enct = []
                for ecn in range(EC):
                    tp = tp_ps.tile([128, ST], f32r, tag="tp", name="tp")
                    for j in range(NSUB):
                        nc.tensor.transpose(
                            tp[:, j * 128:(j + 1) * 128],
                            subs[j][:, ecn * 128:(ecn + 1) * 128],
                            ident,
                        )
                    et = enct_pool.tile([128, ST], f32r, tag="et", name="et")
                    if ecn % 4 == 0:  # ACT is tanh-heavy; DVE takes most copies
                        nc.scalar.copy(out=et, in_=tp)
                    else:
                        nc.vector.tensor_copy(out=et, in_=tp)
                    enct.append(et)
                return enct

            # tile(0,0)'s loads + transposes first: PE has work ~8us in,
            # while the weight DMAs (below) stream.
            subs_cache = {(0, 0): load_subs(0, 0, pfx="00")}

            wbt_sb = []
            for dk in range(DC):
                t = wbt_pool.tile([128, D], f32r, tag="wbt_sb", bufs=4, name=f"wbt{dk}")
                nc.gpsimd.dma_start(out=t, in_=wbt[dk * 128:(dk + 1) * 128, :])
                wbt_sb.append(t)
            dect_sb = []
            for dk in range(DC):
                t = const_pool.tile([128, BL], f32r, tag="dect_sb", bufs=DC, name=f"dect{dk}")
                nc.gpsimd.dma_start(out=t, in_=dect[dk * 128:(dk + 1) * 128, :])
                dect_sb.append(t)
            wct_sb = []
            for ecn in range(EC):
                t = const_pool.tile([128, D], f32r, tag="wct_sb", bufs=EC, name=f"wct{ecn}")
                nc.gpsimd.dma_start(out=t, in_=wct[ecn * 128:(ecn + 1) * 128, :])
                wct_sb.append(t)
            wa_sb = const_pool.tile([128, DC], f32r)
            nc.gpsimd.dma_start(out=wa_sb, in_=wa2[:, :])

            enct_cache = {(0, 0): transpose_tile(subs_cache[(0, 0)])}

            # ---- ws = dec @ Wb.T -> wsT [d2-chunk, b] for the tanh bias ----
            # ws natural [BL, d2]: lhsT = dectChunk [dk, BL], rhs = wbtChunk.
            # dk outer so each wbt chunk is consumed right after its DMA.
            ws_sb = const_pool.tile([BL, D], f32r)
            ws_psum = [wh_ps.tile([BL, 512], f32, tag="wh", name=f"ws_psum{eh}")
                       for eh in range(2)]
            for dk in range(DC):
                for eh in range(2):
                    nc.tensor.matmul(
                        ws_psum[eh], dect_sb[dk], wbt_sb[dk][:, eh * 512:(eh + 1) * 512],
                        start=(dk == 0), stop=(dk == DC - 1),
                    )
            for eh in range(2):
                nc.scalar.copy(out=ws_sb[:, eh * 512:(eh + 1) * 512], in_=ws_psum[eh])
            wst_sb = []
            for dcn in range(DC):
                tp = tp_ps.tile([128, ST], f32r, tag="tp", name="tp_ws")
                nc.tensor.transpose(
                    tp[:, 0:BL], ws_sb[0:BL, dcn * 128:(dcn + 1) * 128], ident[0:BL, 0:BL]
                )
                w = const_pool.tile([128, BL], f32, tag="wst_sb", bufs=DC, name=f"wst{dcn}")
                nc.vector.tensor_copy(out=w, in_=tp[:, 0:BL])
                wst_sb.append(w)

            # ---- main loop over (batch, s-tile) ----
            for bj in range(BL):
                # [32, S] so the row->column move can be a K=32 matmul against
                # e0 (rows 1-31 are zero); only row 0 holds exp scores.
                exp_all = row_pool.tile([32, S], f32, tag="exp_all")
                nc.vector.memset(exp_all, 0.0)
                zparts = row_pool.tile([1, NST], f32, tag="zparts")
                ctx = [ctx_ps.tile([1, 512], f32, tag="ctx", name=f"ctx{eh}")
                       for eh in range(2)]

                for st in range(NST):
                    s0 = st * ST
                    subs = subs_cache.pop((bj, st), None) or load_subs(bj, st)
                    enct = enct_cache.pop((bj, st), None) or transpose_tile(subs)

                    # WH^T + tanh + score, d-chunks in pairs (2 psum banks)
                    sc = sc_ps.tile([1, ST], f32, tag="sc")
                    for dp in range(DC // 2):
                        whs = [wh_ps.tile([128, ST], f32, tag="wh", name=f"wh{dd}")
                               for dd in range(2)]
                        for ecn in range(EC):
                            for dd in range(2):
                                dcn = dp * 2 + dd
                                nc.tensor.matmul(
                                    whs[dd],
                                    wct_sb[ecn][:, dcn * 128:(dcn + 1) * 128],
                                    enct[ecn],
                                    start=(ecn == 0), stop=(ecn == EC - 1),
                                )
                        for dd in range(2):
                            dcn = dp * 2 + dd
                            th = tanh_pool.tile([128, ST], f32r, tag="th", name="th")
                            nc.scalar.activation(
                                out=th, in_=whs[dd], func=TANH,
                                bias=wst_sb[dcn][:, bj:bj + 1], scale=1.0,
                            )
                            nc.tensor.matmul(
                                sc, wa_sb[:, dcn:dcn + 1], th,
                                start=(dcn == 0), stop=(dcn == DC - 1),
                                skip_group_check=True,
                            )

                    # exp (+ per-tile partial of Z via accum_out)
                    nc.scalar.activation(
                        out=exp_all[0:1, s0:s0 + ST], in_=sc, func=EXP,
                        accum_out=zparts[0:1, st:st + 1],
                    )

                    # exp row -> columns [128, 1]: K=32 fp32 matmul with e0
                    # (rows 1-31 of exp_all are zero, so out = row 0)
                    ex = ex_ps.tile([128, NSUB], f32, tag="ex")
                    for j in range(NSUB):
                        nc.tensor.matmul(
                            ex[:, j:j + 1],
                            exp_all[0:32, s0 + j * 128:s0 + (j + 1) * 128],
                            id32[0:32, 0:1],
                            start=True, stop=True,
                        )
                    ext = row_pool.tile([128, NSUB], f32r, tag="ext", bufs=3)
                    nc.vector.tensor_copy(out=ext, in_=ex)

                    # unnormalized context accumulation over all s of this b
                    for j in range(NSUB):
                        for eh in range(2):
                            nc.tensor.matmul(
                                ctx[eh],
                                ext[:, j:j + 1],
                                subs[j][:, eh * 512:(eh + 1) * 512],
                                start=(st == 0 and j == 0),
                                stop=(st == NST - 1 and j == NSUB - 1),
                                skip_group_check=True,
                            )

                # ---- finish batch: scale by 1/Z, write out ----
                z = row_pool.tile([1, 1], f32, tag="z")
                nc.vector.reduce_sum(out=z, in_=zparts, axis=X)
                rz = row_pool.tile([1, 1], f32, tag="rz")
                nc.vector.reciprocal(out=rz, in_=z)
                ctx_sb = row_pool.tile([1, E2], f32, tag="ctx_sb")
                for eh in range(2):
                    nc.vector.tensor_scalar_mul(
                        out=ctx_sb[0:1, eh * 512:(eh + 1) * 512],
                        in0=ctx[eh], scalar1=rz,
                    )
                nc.sync.dma_start(out=outp[bj:bj + 1, :], in_=ctx_sb)

    nc.finalize()
    return nc


def _prep_inputs(dec_prev_hidden, enc_outputs, Wb, Wc, Wa):
    dec_prev_hidden = np.ascontiguousarray(np.asarray(dec_prev_hidden, dtype=np.float32))
    enc_outputs = np.ascontiguousarray(np.asarray(enc_outputs, dtype=np.float32))
    Wb = np.asarray(Wb, dtype=np.float32)
    Wc = np.asarray(Wc, dtype=np.float32)
    Wa = np.asarray(Wa, dtype=np.float32)

    wbt = np.ascontiguousarray(Wb.T)                     # [d, d2]
    wct = np.ascontiguousarray(Wc.T)                     # [e, d]
    dect = np.ascontiguousarray(dec_prev_hidden.T)       # [D, B]
    wa2 = np.ascontiguousarray(Wa.reshape(DC, 128).T)    # [128, DC]

    in_maps = []
    for i in range(NCORES):
        bsl = slice(i * BL, (i + 1) * BL)
        in_maps.append({
            "enc": np.ascontiguousarray(enc_outputs[:, bsl, :]),
            "dect": np.ascontiguousarray(dect[:, bsl]),
            "wbt": wbt,
            "wct": wct,
            "wa2": wa2,
        })
    return in_maps


def _run(inputs, trace=False):
    from concourse.bass_utils import run_bass_kernel_spmd

    if "nc" not in _CACHE:
        _CACHE["nc"] = _build_nc()
    nc = _CACHE["nc"]
    in_maps = _prep_inputs(**inputs)
    res = run_bass_kernel_spmd(nc, in_maps, list(range(NCORES)), trace=trace)
    out = np.concatenate([res.results[i]["out"] for i in range(NCORES)], axis=0)
    return out[None, :, :].astype(np.float32), res


def kernel(dec_prev_hidden, enc_outputs, Wb, Wc, Wa):
    out, _ = _run(dict(
        dec_prev_hidden=dec_prev_hidden, enc_outputs=enc_outputs,
        Wb=Wb, Wc=Wc, Wa=Wa,
    ))
    return out
